# revision 44
# baseline (speedup 1.0000x reference)
"""Trainium2 Bass kernel for nn_MultiHeadAttention_5308579578426.

Multi-head attention, B=2 L=4096 D=512 H=8 DK=DV=64, returning both the
projected output [B, L, DV] and the full attention matrix [H*B, L, L].

Sharding (data + head parallel, no cross-device comm):
  core d -> batch b = d // 4, heads {2*(d%4), 2*(d%4)+1}.
The host pre-transposes q/k/v per batch ([D, L]) and pre-scales the Q
weights by 1/sqrt(DK); each core writes its two heads' attention rows
straight to HBM plus a per-head partial fc output that the host sums.

Per-core algorithm:
  prologue: DMA qT/kT/vT, project with fp32r matmuls into fp16 packed
            layouts: QTd (rows 0-63 = Q^T, 64-127 duplicated) and KTi
            (even lk banks in rows 0-63, odd banks in 64-127), V | ones.
  T-side  : per (head, 512-wide lq group): row-packed fp16 scoresT pairs
            (two concurrent K=64 matmuls in opposite PE array halves),
            ACT exp, then ctx^T accumulation whose ones column yields
            S = rowsum(exp) in psum row 64. From S: invS / -lnS
            (PE-transposed), ctx^T normalization, and the fc projection.
  N-side  : per (head, 128-row lq tile): row-packed natural scores, then
            ACT exp with per-partition bias -lnS -> normalized attention
            rows DMA'd to HBM. (The first groups instead use exp+accum and
            DVE normalize so ACT has work before V/T results exist.)

The container's walrus accepts one sync-wait per instruction, so a post
pass splits Tile's multi-wait instructions into wait-carrying no-ops.
"""

import os
import time
from contextlib import ExitStack

import numpy as np

B, L, D, H, DK, DV = 2, 4096, 512, 8, 64, 64
P = 128
NCORES = 8
HPC = 2  # heads per core
TEMP = 8.0  # sqrt(DK)

_RUNNER_CACHE = {}
LAST_EXEC_S = None  # wall time of the device execution of the last kernel() call


def _fills(total, width):
    """Split `total` into chunks of `width` (last chunk may be smaller)."""
    out = []
    base = 0
    while base < total:
        w = min(width, total - base)
        out.append((base, w))
        base += w
    return out


def _split_multi_waits(nc):
    """The container's walrus accepts at most one sync-wait per instruction;
    Tile attaches several. Hoist all but the last wait of each instruction
    onto no-op instructions inserted immediately before it (same engine, same
    program point -- semantically identical, sequencer waits serially)."""
    from concourse import mybir

    n = 0
    for fn in nc.m.functions:
        for bb in fn.blocks:
            insts = list(bb.instructions)
            if not any(
                i.sync_info is not None and len(i.sync_info.on_wait) > 1
                for i in insts
            ):
                continue
            out = []
            for inst in insts:
                si = inst.sync_info
                if si is not None and len(si.on_wait) > 1:
                    waits = list(si.on_wait)
                    for w in waits[:-1]:
                        n += 1
                        out.append(
                            mybir.InstNoOp(
                                name=f"WSPLIT-{nc.next_id()}",
                                engine=inst.engine,
                                sync_info=mybir.SyncInfo(on_wait=[w], on_update=[]),
                                bass_nofuse=True,
                            )
                        )
                    inst.sync_info = mybir.SyncInfo(
                        on_wait=[waits[-1]], on_update=list(si.on_update)
                    )
                out.append(inst)
            bb.instructions = out
    return n


def _build_nc(L_=L, ld_bufs=2, attn_bufs=4, exp_bufs=3, trn_bufs=2,
              act_groups=0, v_inter=True, n_pre_groups=2, split_waits=True):
    import concourse.bass as bass
    import concourse.tile as tile
    from concourse import mybir
    from concourse.masks import make_identity
    from concourse.bass import ts, ds

    f32 = mybir.dt.float32
    f32r = mybir.dt.float32r
    f16 = mybir.dt.float16
    EXP = mybir.ActivationFunctionType.Exp
    LN = mybir.ActivationFunctionType.Ln

    assert L_ % 1024 == 0
    nT = L_ // P  # lq tiles per head
    nG = L_ // 512  # 512-wide lq groups
    nC = L_ // P  # lk chunks
    nE = nG // 2  # 1024-wide bank pairs

    nc = bass.Bass(
        "TRN2",
        target_bir_lowering=False,
        debug=False,
        enable_asserts=False,
        num_devices=NCORES,
    )

    # q/k/v arrive pre-transposed from the host: [D, L] row-major
    q_d = nc.dram_tensor("qt", [D, L_], f32r, kind="ExternalInput").ap()
    k_d = nc.dram_tensor("kt", [D, L_], f32r, kind="ExternalInput").ap()
    v_d = nc.dram_tensor("vt", [D, L_], f32r, kind="ExternalInput").ap()
    wqt_d = nc.dram_tensor("wqt", [D, HPC * DK], f32r, kind="ExternalInput").ap()
    wkt_d = nc.dram_tensor("wkt", [D, HPC * DK], f32r, kind="ExternalInput").ap()
    wvt_d = nc.dram_tensor("wvt", [D, HPC * DV], f32r, kind="ExternalInput").ap()
    bq_d = nc.dram_tensor("bq", [HPC * DK, 1], f32, kind="ExternalInput").ap()
    bk_d = nc.dram_tensor("bk", [HPC * DK, 1], f32, kind="ExternalInput").ap()
    bv_d = nc.dram_tensor("bv", [1, HPC * DV], f32r, kind="ExternalInput").ap()
    fcwt_d = nc.dram_tensor("fcwt", [HPC * DV, DV], f32r, kind="ExternalInput").ap()

    attn_d = nc.dram_tensor("attn", [HPC, L_, L_], f32, kind="ExternalOutput").ap()
    outp_d = nc.dram_tensor("outp", [HPC, DV, L_], f32, kind="ExternalOutput").ap()

    with tile.TileContext(nc) as tc, ExitStack() as ctx:
        sync, vec, act, pe = nc.sync, nc.vector, nc.scalar, nc.tensor

        const = ctx.enter_context(tc.tile_pool(name="const", bufs=1))
        main = ctx.enter_context(tc.tile_pool(name="main", bufs=1))
        trn = ctx.enter_context(tc.tile_pool(name="trn", bufs=trn_bufs))
        attnp = ctx.enter_context(tc.tile_pool(name="attnp", bufs=attn_bufs))
        expp = ctx.enter_context(tc.tile_pool(name="expp", bufs=exp_bufs))
        stat = ctx.enter_context(tc.tile_pool(name="stat", bufs=8))
        ps = ctx.enter_context(tc.tile_pool(name="ps", bufs=3, space="PSUM"))
        misc = ctx.enter_context(tc.tile_pool(name="misc", bufs=2, space="PSUM"))

        ident = const.tile([P, P], f32, tag="ident")
        make_identity(nc, ident)
        ones1 = const.tile([1, P], f32r, tag="ones1")
        vec.memset(ones1.bitcast(f32), 1.0)
        ones1h = const.tile([1, P], f16, tag="ones1h")
        vec.memset(ones1h, 1.0)

        # --- weights to SBUF ---
        wq_sb = const.tile([P, 4, HPC * DK], f32r, tag="wq")
        sync.dma_start(out=wq_sb, in_=wqt_d.rearrange("(a p) h -> p a h", p=P))
        wk_sb = const.tile([P, 4, HPC * DK], f32r, tag="wk")
        sync.dma_start(out=wk_sb, in_=wkt_d.rearrange("(a p) h -> p a h", p=P))
        wv_sb = const.tile([P, 4, HPC * DV], f32r, tag="wv")
        sync.dma_start(out=wv_sb, in_=wvt_d.rearrange("(a p) h -> p a h", p=P))
        bq_sb = const.tile([DK, HPC], f32, tag="bq")
        sync.dma_start(out=bq_sb, in_=bq_d.rearrange("(h a) x -> a (h x)", h=HPC))
        bk_sb = const.tile([DK, HPC], f32, tag="bk")
        sync.dma_start(out=bk_sb, in_=bk_d.rearrange("(h a) x -> a (h x)", h=HPC))
        bv_sb = const.tile([1, HPC * DV], f32r, tag="bv")
        sync.dma_start(out=bv_sb, in_=bv_d)
        fcw_sb = const.tile([DV, HPC, DV], f32r, tag="fcw")
        sync.dma_start(out=fcw_sb, in_=fcwt_d.rearrange("(h a) d -> a h d", h=HPC))
        wv_sbh = const.tile([P, 4, HPC * DV], f16, tag="wvh")
        vec.tensor_copy(out=wv_sbh, in_=wv_sb.bitcast(f32))
        bv_sbh = const.tile([1, HPC * DV], f16, tag="bvh")
        vec.tensor_copy(out=bv_sbh, in_=bv_sb.bitcast(f32))

        # --- persistent per-head tensors (fp16, row-pack friendly) ---
        # QTd[h]: [128, L]; rows 0..63 = Q^T, rows 64..127 = a copy, so the
        #   idle half of the PE array can run a second K=64 matmul.
        # KTi[h]: [128, L/2]; rows 0..63 hold the EVEN 512-wide lk banks,
        #   rows 64..127 the ODD banks (bank pair e at free ts(e, 512)).
        # V_sb: [lk-part, chunk, head*(DV+2)] -- per head [V | ones | ones];
        #   the ones column makes the ctx matmul accumulate S in psum row 64.
        QTd = [main.tile([P, L_], f16, tag=f"qtd{h}", name=f"qtd{h}") for h in range(HPC)]
        KTi = [main.tile([P, L_ // 2], f16, tag=f"kti{h}", name=f"kti{h}") for h in range(HPC)]
        V_sb = main.tile([P, nC, HPC * (DV + 2)], f16, tag="vsb")
        vec.memset(V_sb, 1.0)
        cstage = ctx.enter_context(tc.tile_pool(name="cstage", bufs=2))
        ostage = ctx.enter_context(tc.tile_pool(name="ostage", bufs=2))

        # [D, L] -> [p = d%128, dc = d//128, lq]
        q_r = q_d.rearrange("(a p) l -> p a l", p=P)
        k_r = k_d.rearrange("(a p) l -> p a l", p=P)
        v_r = v_d.rearrange("(a p) l -> p a l", p=P)

        def load_group(src_r, g):
            """Load the transposed-layout group [128, 4, 512]: partition = d
            within chunk dc, free = lq in the 512-wide group g."""
            gw = min(4, nT - 4 * g)
            tg = trn.tile([P, 4, 512], f32r, tag="trn")
            sync.dma_start(
                out=tg[:, :, : gw * P], in_=src_r[:, :, ds(g * 512, gw * P)]
            )
            return tg, gw

        def proj_group(src_r, g, w_sb, b_sb, kind):
            """Project one 512-wide lq/lk group for both heads into the fp16
            packed layouts (kind='q' -> QTd + dup, kind='k' -> KTi)."""
            tg, gw = load_group(src_r, g)
            for h in range(HPC):
                pool_, tag_ = (misc, "misc") if (g + h) % 2 == 0 else (ps, "ps")
                pp = pool_.tile([DK, 512], f32, tag=tag_, name="pp")
                for dc in range(4):
                    pe.matmul(
                        pp[:, : gw * P],
                        lhsT=w_sb[:, dc, h * DK : (h + 1) * DK],
                        rhs=tg[:, dc, : gw * P],
                        start=(dc == 0),
                        stop=(dc == 3),
                    )
                if kind == "q":
                    vec.tensor_scalar_add(
                        out=QTd[h][0:DK, ds(g * 512, gw * P)],
                        in0=pp[:, : gw * P],
                        scalar1=b_sb[:, h : h + 1],
                    )
                    # duplicate into rows 64..127 (cross-partition -> DMA)
                    sync.dma_start(
                        out=QTd[h][DK:P, ds(g * 512, gw * P)],
                        in_=QTd[h][0:DK, ds(g * 512, gw * P)],
                    )
                else:
                    vec.tensor_scalar_add(
                        out=KTi[h][
                            (g % 2) * DK : (g % 2) * DK + DK,
                            ds((g // 2) * 512, gw * P),
                        ],
                        in0=pp[:, : gw * P],
                        scalar1=b_sb[:, h : h + 1],
                    )

        def v_group(g):
            tg, gw = load_group(v_r, g)
            tgh = trn.tile([P, 4, 512], f16, tag="trnh")
            vec.tensor_copy(out=tgh[:, :, : gw * P], in_=tg[:, :, : gw * P].bitcast(f32))
            for tt in range(gw):
                c = 4 * g + tt
                pool_, tag_ = (misc, "misc") if c % 2 == 0 else (ps, "ps")
                pv = pool_.tile([P, HPC * DV], f32, tag=tag_, name="pv")
                for dc in range(4):
                    pe.matmul(
                        pv,
                        lhsT=tgh[:, dc, ts(tt, P)],
                        rhs=wv_sbh[:, dc, :],
                        start=(dc == 0),
                        stop=False,
                    )
                pe.matmul(
                    pv,
                    lhsT=ones1h,
                    rhs=bv_sbh,
                    start=False,
                    stop=True,
                )
                for h in range(HPC):
                    vec.tensor_copy(
                        out=V_sb[:, c, h * (DV + 2) : h * (DV + 2) + DV],
                        in_=pv[:, h * DV : (h + 1) * DV],
                    )

        for g in range((nT + 3) // 4):
            proj_group(k_r, g, wk_sb, bk_sb, "k")
            proj_group(q_r, g, wq_sb, bq_sb, "q")
            if v_inter:
                v_group(g)
        if not v_inter:
            for g in range((nT + 3) // 4):
                v_group(g)

        def scores_nat_fill(h, t, e, pn):
            """Packed pair of natural-score matmuls: lk banks 2e (rows 0-63)
            and 2e+1 (rows 64-127) -> pn[:, 0:512] / pn[:, 512:1024]."""
            pe.matmul(
                pn[:, 0:512],
                lhsT=QTd[h][0:DK, ts(t, P)],
                rhs=KTi[h][0:DK, ts(e, 512)],
                start=True,
                stop=True,
            )
            pe.matmul(
                pn[:, 512:1024],
                lhsT=QTd[h][DK:P, ts(t, P)],
                rhs=KTi[h][DK:P, ts(e, 512)],
                start=True,
                stop=True,
            )

        def emit_t_group(h, g):
            """T-side: packed scoresT pairs, exp, ctx^T + S via the ones
            column, normalize ctx^T, fc, and -lnS columns for the N-side."""
            pc = misc.tile([DV + 2, 512], f32, tag="misc", name="pc")
            first = True
            for e in range(nE):
                for i in range(4):
                    cA = 8 * e + i        # chunk in even bank 2e
                    cB = 8 * e + 4 + i    # chunk in odd bank 2e+1
                    pt = ps.tile([P, 1024], f32, tag="ps", name="pt")
                    et = expp.tile([P, 1024], f16, tag="expp", name="et")
                    pe.matmul(
                        pt[:, 0:512],
                        lhsT=KTi[h][0:DK, ds(e * 512 + i * P, P)],
                        rhs=QTd[h][0:DK, ts(g, 512)],
                        start=True,
                        stop=True,
                    )
                    pe.matmul(
                        pt[:, 512:1024],
                        lhsT=KTi[h][DK:P, ds(e * 512 + i * P, P)],
                        rhs=QTd[h][DK:P, ts(g, 512)],
                        start=True,
                        stop=True,
                    )
                    act.activation(out=et, in_=pt, func=EXP)
                    last = e == nE - 1 and i == 3
                    pe.matmul(
                        pc,
                        lhsT=V_sb[:, cA, h * (DV + 2) : (h + 1) * (DV + 2)],
                        rhs=et[:, 0:512],
                        start=first,
                        stop=False,
                    )
                    pe.matmul(
                        pc,
                        lhsT=V_sb[:, cB, h * (DV + 2) : (h + 1) * (DV + 2)],
                        rhs=et[:, 512:1024],
                        start=False,
                        stop=last,
                    )
                    first = False
            # stage ctx^T + S to SBUF; derive invS (transposed), -lnS, and
            # the broadcast row for normalization
            cs = cstage.tile([DV + 1, 512], f32, tag="cstage", name="cs")
            vec.tensor_copy(out=cs, in_=pc[0 : DV + 1, :])
            pst = misc.tile([P, 4], f32, tag="misc", name="pst")
            for j in range(4):
                pe.transpose(
                    out=pst[:, j : j + 1],
                    in_=cs[DV : DV + 1, ts(j, P)],
                    identity=ident[DV : DV + 1, DV : DV + 1],
                )
            invS_t = stat.tile([P, 4], f32, tag="invs", bufs=4)
            vec.reciprocal(out=invS_t, in_=pst)
            nlnS = stat.tile([P, 4], f32, tag="nlns", bufs=4)
            act.activation(out=nlnS, in_=invS_t, func=LN)
            prow = misc.tile([1, 512], f32, tag="misc", name="prow")
            for j in range(4):
                pe.transpose(
                    out=prow[0:1, ts(j, P)], in_=invS_t[:, j : j + 1], identity=ident
                )
            invS_row = stat.tile([1, 512], f32r, tag="invsrow", bufs=2)
            vec.tensor_copy(out=invS_row, in_=prow)
            pb = misc.tile([P, 512], f32, tag="misc", name="pb")
            pe.matmul(pb, lhsT=ones1, rhs=invS_row, start=True, stop=True)
            cnorm = cstage.tile([DV, 512], f32r, tag="cstage", name="cnorm")
            vec.tensor_mul(cnorm, cs[0:DV, :], pb[0:DV, :])
            # fc: out^T = fcw_h.T @ ctx^T
            pf = misc.tile([DV, 512], f32, tag="misc", name="pf")
            pe.matmul(pf, lhsT=fcw_sb[:, h, :], rhs=cnorm, start=True, stop=True)
            osb = ostage.tile([DV, 512], f32, tag="ostage", name="osb")
            vec.tensor_copy(out=osb, in_=pf)
            sync.dma_start(out=outp_d[h, :, ts(g, 512)], in_=osb)
            return nlnS

        def emit_n_tile_v1(h, t):
            """Self-sufficient N-side tile (no T-group dependency): raw exp
            with fused row-sum, then DVE normalize. Used for the first groups
            so ACT has work while the V prologue finishes."""
            at = attnp.tile([P, L_], f32, tag="attn", name="at")
            sp = stat.tile([P, 8], f32, tag="stat", bufs=4)
            for e in range(nE):
                pn = ps.tile([P, 1024], f32, tag="ps", name="pn")
                scores_nat_fill(h, t, e, pn)
                act.activation(
                    out=at[:, ds(e * 1024, 1024)],
                    in_=pn,
                    func=EXP,
                    accum_out=sp[:, e : e + 1],
                )
            if nE == 1:
                vec.reciprocal(sp[:, 5:6], sp[:, 0:1])
            else:
                vec.tensor_add(sp[:, 4:5], sp[:, 0:1], sp[:, 1:2])
                for e in range(2, nE):
                    vec.tensor_add(sp[:, 4:5], sp[:, 4:5], sp[:, e : e + 1])
                vec.reciprocal(sp[:, 5:6], sp[:, 4:5])
            vec.tensor_scalar_mul(out=at, in0=at, scalar1=sp[:, 5:6])
            sync.dma_start(out=attn_d[h, ts(t, P), :], in_=at)

        def emit_n_tile(h, t, nlnS):
            """N-side: packed natural scores, then exp(scores - lnS) via the
            ACT per-partition bias -> normalized attention rows, to HBM."""
            j = t % 4
            at = attnp.tile([P, L_], f32, tag="attn", name="at")
            for e in range(nE):
                pn = ps.tile([P, 1024], f32, tag="ps", name="pn")
                scores_nat_fill(h, t, e, pn)
                act.activation(
                    out=at[:, ds(e * 1024, 1024)],
                    in_=pn,
                    func=EXP,
                    bias=nlnS[:, j : j + 1],
                )
            sync.dma_start(out=attn_d[h, ts(t, P), :], in_=at)

        # First groups' N tiles are self-sufficient (v1 style) so ACT has
        # work while the V prologue finishes; afterwards T group first
        # (produces -lnS), then its four bias-normalized N tiles.
        n_pre = min(n_pre_groups, nG)
        for h in range(HPC):
            for t in range(min(4 * n_pre, nT)):
                emit_n_tile_v1(h, t)
        for g in range(nG):
            for h in range(HPC):
                nlnS = emit_t_group(h, g)
                if g >= n_pre:
                    for t in range(4 * g, min(4 * g + 4, nT)):
                        emit_n_tile(h, t, nlnS)

    if split_waits:
        _split_multi_waits(nc)
    return nc


def _shard_inputs(inputs):
    """Full inputs -> per-core input maps (host-side, cheap)."""
    q, k, v = inputs["q"], inputs["k"], inputs["v"]
    wq, bq, wk, bk = inputs["wq"], inputs["bq"], inputs["wk"], inputs["bk"]
    wv, bv, fc_w = inputs["wv"], inputs["bv"], inputs["fc_w"]
    f32 = np.float32
    # pre-transpose per batch once (shared by the 4 cores of that batch)
    qT = [np.ascontiguousarray(q[b].T, dtype=f32) for b in range(B)]
    kT = [np.ascontiguousarray(k[b].T, dtype=f32) for b in range(B)]
    vT = [np.ascontiguousarray(v[b].T, dtype=f32) for b in range(B)]
    in_maps = []
    for d in range(NCORES):
        b = d // 4
        h0 = HPC * (d % 4)
        rows = slice(h0 * DK, (h0 + HPC) * DK)
        in_maps.append(
            {
                "qt": qT[b],
                "kt": kT[b],
                "vt": vT[b],
                "wqt": np.ascontiguousarray((wq[rows] / TEMP).T, dtype=f32),
                "wkt": np.ascontiguousarray(wk[rows].T, dtype=f32),
                "wvt": np.ascontiguousarray(wv[rows].T, dtype=f32),
                "bq": np.ascontiguousarray(
                    (bq[rows] / TEMP).reshape(HPC * DK, 1), dtype=f32
                ),
                "bk": np.ascontiguousarray(bk[rows].reshape(HPC * DK, 1), dtype=f32),
                "bv": np.ascontiguousarray(bv[rows].reshape(1, HPC * DV), dtype=f32),
                "fcwt": np.ascontiguousarray(fc_w[:, rows].T, dtype=f32),
            }
        )
    return in_maps


def _get_runner():
    """Build the Bass module once and return a cached jitted SPMD callable."""
    if "runner" in _RUNNER_CACHE:
        return _RUNNER_CACHE["runner"]

    import jax
    from jax.sharding import Mesh, PartitionSpec, NamedSharding

    try:
        from jax.experimental.shard_map import shard_map
    except ImportError:  # newer jax
        shard_map = jax.shard_map
    from concourse import bass2jax, mybir

    bass2jax.install_neuronx_cc_hook()
    nc = _build_nc()

    partition_name = nc.partition_id_tensor.name if nc.partition_id_tensor else None
    in_names, out_names, out_avals, out_shapes = [], [], [], []
    for alloc in nc.m.functions[0].allocations:
        if not isinstance(alloc, mybir.MemoryLocationSet):
            continue
        if alloc.kind not in ("ExternalInput", "ExternalOutput"):
            continue
        name = alloc.memorylocations[0].name
        if alloc.kind == "ExternalInput":
            if name != partition_name:
                in_names.append(name)
        else:
            out_names.append(name)
            shape = tuple(alloc.tensor_shape)
            dtype = mybir.dt.np(alloc.dtype)
            out_avals.append(jax.core.ShapedArray(shape, dtype))
            out_shapes.append((shape, dtype))
    n_params = len(in_names)
    all_in_names = list(in_names) + list(out_names)
    if partition_name is not None:
        all_in_names.append(partition_name)
    all_in_names = tuple(all_in_names)
    donate = tuple(range(n_params, n_params + len(out_names)))

    def _body(*args):
        operands = list(args)
        if partition_name is not None:
            operands.append(bass2jax.partition_id_tensor())
        outs = bass2jax._bass_exec_p.bind(
            *operands,
            out_avals=tuple(out_avals),
            in_names=all_in_names,
            out_names=tuple(out_names),
            lowering_input_output_aliases=(),
            sim_require_finite=True,
            sim_require_nnan=True,
            nc=nc,
        )
        return tuple(outs)

    devices = jax.devices()[:NCORES]
    assert len(devices) == NCORES, f"need {NCORES} cores, got {len(jax.devices())}"
    mesh = Mesh(np.asarray(devices), ("core",))
    in_specs = (PartitionSpec("core"),) * (n_params + len(out_names))
    out_specs = (PartitionSpec("core"),) * len(out_names)
    fn = jax.jit(
        shard_map(
            _body, mesh=mesh, in_specs=in_specs, out_specs=out_specs, check_rep=False
        ),
        donate_argnums=donate,
        keep_unused=True,
    )
    sharding = NamedSharding(mesh, PartitionSpec("core"))

    # on-device zero buffers for the donated outputs (kernel writes every
    # element, but the custom-call path wants donated operands to reuse)
    def _zeros():
        import jax.numpy as jnp

        return tuple(
            jnp.zeros((NCORES * s[0],) + tuple(s[1:]), dt) for (s, dt) in out_shapes
        )

    zeros_fn = jax.jit(_zeros, out_shardings=(sharding,) * len(out_shapes))

    runner = (jax, fn, zeros_fn, in_names, out_names, sharding)
    _RUNNER_CACHE["runner"] = runner
    return runner


def kernel(**inputs):
    global LAST_EXEC_S
    jax, fn, zeros_fn, in_names, out_names, sharding = _get_runner()

    in_maps = _shard_inputs(inputs)
    concat = [
        jax.device_put(
            np.concatenate([in_maps[c][nm] for c in range(NCORES)], axis=0), sharding
        )
        for nm in in_names
    ]
    zouts = zeros_fn()
    jax.block_until_ready((concat, zouts))

    t0 = time.perf_counter()
    outs = fn(*concat, *zouts)
    outs = jax.block_until_ready(outs)
    LAST_EXEC_S = time.perf_counter() - t0

    res = {nm: np.asarray(o) for nm, o in zip(out_names, outs)}
    # attn: [8*2, L, L]; core d rows [2d, 2d+1] are heads (2*(d%4), 2*(d%4)+1)
    # of batch d//4. attn_view[h*B + b] = global row 8*b + h.
    ga = res["attn"]
    idx = [8 * b + h for h in range(H) for b in range(B)]
    attn_view = ga[idx]
    # outp: per-core [HPC, DV, L] partial outT per head
    go = res["outp"].reshape(NCORES, HPC, DV, L)
    fc_b = np.asarray(inputs["fc_b"], dtype=np.float32)
    out = np.empty((B, L, DV), dtype=np.float32)
    for b in range(B):
        acc = go[4 * b : 4 * b + 4].sum(axis=(0, 1))  # [DV, L]
        out[b] = acc.T + fc_b
    return out, attn_view
